# revision 43
# baseline (speedup 1.0000x reference)
"""Trainium2 Bass kernel for nn_AttentionCroiseeVariables.

Reference computation (N=4 vars, B=4, T=512, D=512, H=8, DK=DV=64):
  q,k,v = per-var projections of x; all-pairs (q_var, k_var) attention with
  per-key-var softmax; per-pair output projection; mean over key vars;
  residual + LayerNorm.

Sharding: 8 cores = (B=4) x (T split in 2 halves of 256 query tokens).
Core ci handles b = ci // 2, query-token half th = ci % 2.  Each core
computes its queries' attention over ALL key/value vars at full T=512.

Key optimizations over the plain-bf16 version:
  - Block-diagonal packing per head-pair: kt is stored as [128 dk-pair,
    (64 keys | same 64 keys)] block-diagonal chunks so every score matmul
    runs at K=128 and lands both heads' scores for a 64-key chunk in one
    [128, t] PSUM tile (rows 0-63 head even, 64-127 head odd).
  - attn weights and V are fp8e4 (e4m3); the AV matmuls AND the
    ones-denominator matmuls use block-diagonal stationaries in DoubleRow
    perf mode (2 key-chunks per pass) at 2x PE throughput, writing
    [128, 512] PSUM at partition base 0 (a DR ISA requirement).
    exp(s/8) <= ~e^5.5 = 245 < 448 fits e4m3; weights under ~2e-3 flush,
    negligible vs the ~e2-e3 denominators.
  - out-projection pre-sums ctx over the 4 key-vars (linearity) before
    the Wo matmul: 16 -> 4 matmuls per token block.
  - x is host-permuted so the core's query tokens are the first 1024
    columns of xt: the separate xq load disappears.
  - bo is folded into xres host-side.
  - [1,512] vector loads (bv/gamma/beta) are broadcast across partitions
    on-chip with a ones[1,128] matmul instead of 256KB bcast DMAs.
  - input DMAs round-robin over the 3 DGE queues, ordered xt -> wk/wq ->
    wv -> rest so the first score block unblocks much earlier.
"""

import sys

import numpy as np

try:
    import concourse.bass as bass  # noqa: F401
except Exception:  # pragma: no cover
    sys.path.insert(0, "/opt/trn_rl_repo")

import ml_dtypes

import concourse.bass as bass
import concourse.tile as tile
from concourse import bacc, mybir
from concourse.bass_utils import run_bass_kernel_spmd

BF = mybir.dt.bfloat16
F32 = mybir.dt.float32
FP8 = mybir.dt.float8e4
FP8E5 = mybir.dt.float8e5
AF = mybir.ActivationFunctionType
OP = mybir.AluOpType
DR = mybir.MatmulPerfMode.DoubleRow

N, B, T, D = 4, 4, 512, 512
H, DK, DV = 8, 64, 64
TH = T // 2          # query tokens per core
NTOK = N * T         # kv tokens per core (all vars, one batch)
LN_EPS = 1e-5
SCALE = 1.0 / np.sqrt(DK)

_NC_CACHE = {}


def _kc_xtcol(c, kc):
    """xt column base of the 64-key chunk kc (0..7) of kv-var c in the
    host-permuted token order [q-half var-major | other-half var-major].
    Returns (half, col) with half 0 = xtA, 1 = xtB."""
    if kc < 4:
        return 0, 256 * c + 64 * kc
    return 1, 256 * c + 64 * (kc - 4)


def build_nc():
    nc = bacc.Bacc(None, target_bir_lowering=False)

    xt_d = nc.dram_tensor("xt", [D, NTOK], BF, kind="ExternalInput")
    xres_d = nc.dram_tensor("xres", [N * TH, D], F32, kind="ExternalInput")
    wq_d = nc.dram_tensor("wq", [D, H * DK], BF, kind="ExternalInput")
    wk_d = nc.dram_tensor("wk", [D, H * DK], BF, kind="ExternalInput")
    wv_d = nc.dram_tensor("wv", [D, H * DV], BF, kind="ExternalInput")
    wo_d = nc.dram_tensor("wo", [H * DV, D], BF, kind="ExternalInput")
    bv_d = nc.dram_tensor("bv", [H * DV], F32, kind="ExternalInput")
    out_d = nc.dram_tensor("out", [N * TH, D], BF, kind="ExternalOutput")

    with tile.TileContext(nc) as tc:
        with (
            tc.tile_pool(name="const", bufs=1) as constp,
            tc.tile_pool(name="xt", bufs=1) as xtp,
            tc.tile_pool(name="wts", bufs=1) as wtsp,
            tc.tile_pool(name="qkv", bufs=1) as qkvp,
            tc.tile_pool(name="attn", bufs=4) as attnp,
            tc.tile_pool(name="rbp", bufs=2) as rbp,
            tc.tile_pool(name="ctx", bufs=1) as ctxp,
            tc.tile_pool(name="sums", bufs=1) as sumsp,
            tc.tile_pool(name="outs", bufs=1) as outsp,
            tc.tile_pool(name="fin", bufs=2) as finp,
            tc.tile_pool(name="ps_s", bufs=2, space="PSUM") as ps_s,
            tc.tile_pool(name="ps_d", bufs=1, space="PSUM") as ps_d,
            tc.tile_pool(name="ps_av", bufs=1, space="PSUM") as ps_av,
            tc.tile_pool(name="ps_big", bufs=2, space="PSUM") as ps_big,
        ):
            # ---- DMA round-robin over the 3 DGE queues
            dmaq = [nc.sync, nc.scalar, nc.gpsimd]
            qi = [0]

            def dma(out, in_):
                eng = dmaq[qi[0] % 3]
                qi[0] += 1
                eng.dma_start(out=out, in_=in_)

            # ---- constants (cheap, engine-local)
            ones1 = constp.tile([1, 128], F32)
            nc.vector.memset(ones1, 1.0)
            # block-diagonal ones for the DoubleRow denominator matmuls
            ones_bd = constp.tile([128, 2, 128], FP8)
            nc.vector.memset(ones_bd, 0.0)
            nc.vector.memset(ones_bd[0:64, :, 0:64], 1.0)
            nc.vector.memset(ones_bd[64:128, :, 64:128], 1.0)
            eps_sb = constp.tile([128, 1], F32)
            nc.vector.memset(eps_sb, LN_EPS)

            # ---- critical loads. The first attention block needs
            # wk + wq + the FIRST column-halves of xtA/xtB (vars 0,1), so
            # xt is loaded as 512-col half tiles in criticality order.
            # first halves live in one merged tile each: a single 512KB
            # DMA amortizes the ~2us per-DMA fixed cost on the head path
            xtA1m = xtp.tile([128, 4, 512], BF, tag="xtA1m", name="xtA1m")
            xtB1m = xtp.tile([128, 4, 512], BF, tag="xtB1m", name="xtB1m")
            xtA, xtB = [], []   # [dj] -> (half1, half2)
            for dj in range(4):
                a2 = xtp.tile([128, 512], BF, tag=f"xtA2{dj}", name=f"xtA2{dj}")
                b2 = xtp.tile([128, 512], BF, tag=f"xtB2{dj}", name=f"xtB2{dj}")
                xtA.append((xtA1m[:, dj, :], a2))
                xtB.append((xtB1m[:, dj, :], b2))
            wq_sb, wv_sb, wo_sb = [], [], []
            for lst, nm in ((wq_sb, "wq"), (wv_sb, "wv"), (wo_sb, "wo")):
                for dj in range(4):
                    lst.append(
                        wtsp.tile([128, 512], BF, tag=f"{nm}{dj}", name=f"{nm}{dj}")
                    )
            wkj_sb = [
                [
                    wtsp.tile([128, 128], BF, tag=f"wk{j}_{dj}", name=f"wk{j}_{dj}")
                    for dj in range(4)
                ]
                for j in range(4)
            ]
            bv1 = constp.tile([1, 512], F32)
            for dj in range(4):
                dma(wkj_sb[0][dj], wk_d[128 * dj : 128 * (dj + 1), 0:128])
            dma(bv1, bv_d[:])
            dma(xtA1m, xt_d[:, 0:512].rearrange("(dj p) t -> p dj t", p=128))
            for dj in range(4):
                dma(wq_sb[dj], wq_d[128 * dj : 128 * (dj + 1), :])
            dma(xtB1m, xt_d[:, 1024:1536].rearrange("(dj p) t -> p dj t", p=128))
            for j in range(1, 4):
                for dj in range(4):
                    dma(
                        wkj_sb[j][dj],
                        wk_d[128 * dj : 128 * (dj + 1), 128 * j : 128 * (j + 1)],
                    )
            for dj in range(4):
                dma(xtA[dj][1], xt_d[128 * dj : 128 * (dj + 1), 512:1024])
            for dj in range(4):
                dma(xtB[dj][1], xt_d[128 * dj : 128 * (dj + 1), 1536:2048])
            for dj in range(4):
                dma(wv_sb[dj], wv_d[128 * dj : 128 * (dj + 1), :])
            for dj in range(4):
                dma(wo_sb[dj], wo_d[128 * dj : 128 * (dj + 1), :])
            xres_sb = []
            for r in range(8):
                t_ = outsp.tile([128, D], F32, tag=f"xres{r}", name=f"xres{r}")
                dma(t_, xres_d[128 * r : 128 * (r + 1), :])
                xres_sb.append(t_)

            def bcast_vec(src1):
                ps = ps_big.tile([128, 512], F32, tag="big", name="bc")
                nc.tensor.matmul(ps, ones1, src1, start=True, stop=True)
                sb = constp.tile(
                    [128, 512], F32, name="bcsb", tag=f"bc_{src1.tensor.name}"
                )
                nc.vector.tensor_scalar_mul(sb, ps, 1.0)
                return sb

            bv_box = []

            # ---- attention block, software-pipelined: the scores+exp of
            # block N are emitted before the d/av/normalize of block N-1, so
            # the PE never sits behind the ACT exp of the current block.
            def emit_scores(qvp, c, j):
                # attn8 layout: [k 128 = (64 keys scored by head 2j | same
                # keys scored by head 2j+1)][scp 4][i 2][qh 2][t 256], fp8e5.
                # Key chunk kc = 2*scp + i (64 keys each, 8 chunks = 512).
                attn8 = attnp.tile([128, 4, 2, 2, 256], FP8E5, tag="attn", name="a8")
                for scp in range(4):
                    s = ps_s.tile([128, 1024], F32, tag="s", name="s")
                    for i in range(2):
                        kc = 2 * scp + i
                        half, col = _kc_xtcol(c, kc)
                        kcg = (16 if half else 0) + col // 64
                        nc.tensor.matmul(
                            s[:, 512 * i : 512 * (i + 1)],
                            kt_bd[j][:, kcg, :],
                            qt_sb[j][:, 512 * qvp : 512 * (qvp + 1)],
                            start=True,
                            stop=True,
                        )
                    # dst = attn8[:, scp] is contiguous (i, qv, t) = src
                    nc.scalar.activation(
                        attn8[:, scp, :, :, :],
                        s,
                        AF.Exp,
                        scale=float(SCALE),
                    )
                return attn8

            def emit_dav(qvp, c, j, attn8, ctx_tiles):
                d_ps = ps_d.tile([128, 512], F32, tag="d", name="d")
                av_ps = ps_av.tile([128, 512], F32, tag="av", name="av")
                for scp in range(4):
                    rhs = attn8[:, scp, :, :, :]
                    nc.tensor.matmul(
                        d_ps,
                        ones_bd,
                        rhs,
                        start=(scp == 0),
                        stop=(scp == 3),
                        perf_mode=DR,
                    )
                    nc.tensor.matmul(
                        av_ps,
                        v8_sb[c][scp][:, :, 128 * j : 128 * (j + 1)],
                        rhs,
                        start=(scp == 0),
                        stop=(scp == 3),
                        perf_mode=DR,
                    )
                rb = rbp.tile([128, 512], F32, tag="rb", name="rb")
                nc.vector.reciprocal_approx_fast(rb, d_ps)
                ctx = ctxp.tile([128, 512], BF, tag=f"ctx{c}_{j}", name="ctx")
                nc.vector.tensor_tensor(ctx, av_ps, rb, OP.mult)
                ctx_tiles[(c, j)] = ctx
                # progressive pairwise pre-sum over c (for the out-proj)
                if c == 1:
                    s01 = sumsp.tile([128, 512], BF, tag=f"s01_{j}", name="s01")
                    nc.gpsimd.tensor_tensor(
                        s01, ctx_tiles[(0, j)], ctx_tiles[(1, j)], OP.add
                    )
                    ctx_tiles[("s01", j)] = s01
                elif c == 3:
                    eng = nc.vector if j == 3 else nc.gpsimd
                    s23 = sumsp.tile([128, 512], BF, tag=f"s23_{j}", name="s23")
                    eng.tensor_tensor(
                        s23, ctx_tiles[(2, j)], ctx_tiles[(3, j)], OP.add
                    )
                    cs = sumsp.tile([128, 512], BF, tag=f"cs_{j}", name="cs")
                    eng.tensor_tensor(cs, ctx_tiles[("s01", j)], s23, OP.add)
                    ctx_tiles[("cs", j)] = cs
                    # progressive out-proj: units 0,1 (qh=0) accumulate as
                    # each cs_j lands, spreading Wo matmuls over the last
                    # blocks instead of a serial tail.
                    for u in range(2):
                        if j == 0:
                            ctx_tiles[("o", u)] = ps_big.tile(
                                [128, 512], F32, tag="big", name=f"o{u}"
                            )
                        nc.tensor.matmul(
                            ctx_tiles[("o", u)],
                            cs[:, 128 * u : 128 * (u + 1)],
                            wo_sb[j],
                            start=(j == 0),
                            stop=(j == 3),
                        )

            pending = [None]

            def pump(qvp, c, j, ctx_tiles):
                attn8 = emit_scores(qvp, c, j)
                drain()
                pending[0] = (qvp, c, j, attn8, ctx_tiles)

            def drain():
                if pending[0] is not None:
                    emit_dav(*pending[0])
                    pending[0] = None

            def emit_ln_front(qvp, qh, tch, o_ps):
                r = 4 * qvp + 2 * qh + tch
                res = outsp.tile([128, D], F32, tag=f"res{r}", name="res")
                nc.vector.scalar_tensor_tensor(
                    res, o_ps, 1.0 / N, xres_sb[r], OP.mult, OP.add
                )
                stats = finp.tile([128, 6], F32, tag="stats", name="st")
                nc.vector.bn_stats(stats, res)
                mv = outsp.tile([128, 2], F32, tag=f"mv{r}", name="mv")
                nc.vector.bn_aggr(mv, stats)
                rstd = outsp.tile([128, 1], F32, tag=f"rstd{r}", name="rst")
                nc.scalar.activation(rstd, mv[:, 1:2], AF.Sqrt, bias=eps_sb)
                return res, mv, rstd

            def emit_ln_back(qvp, qh, tch, state):
                r = 4 * qvp + 2 * qh + tch
                res, mv, rstd = state
                rstd2 = finp.tile([128, 1], F32, tag="rstd2", name="rs2")
                nc.vector.reciprocal(rstd2, rstd)
                nmu = finp.tile([128, 1], F32, tag="nmu", name="nmu")
                nc.vector.scalar_tensor_tensor(
                    nmu, mv[:, 0:1], -1.0, rstd2, OP.mult, OP.mult
                )
                y = finp.tile([128, D], BF, tag="y", name="y")
                nc.scalar.activation(
                    y, res, AF.Identity, bias=nmu, scale=rstd2
                )
                if r < 4:  # mid-kernel: keep the exp engine free
                    eng = (nc.sync, nc.gpsimd)[r % 2]
                else:
                    eng = (nc.sync, nc.gpsimd, nc.scalar, nc.gpsimd)[r % 4]
                eng.dma_start(out=out_d[128 * r : 128 * (r + 1), :], in_=y)

            def emit_outproj_ln(qvp, ctx_tiles):
                # units 0,1 (qh=0) were accumulated progressively
                states = {}
                states[(0, 0)] = emit_ln_front(qvp, 0, 0, ctx_tiles[("o", 0)])
                states[(0, 1)] = emit_ln_front(qvp, 0, 1, ctx_tiles[("o", 1)])
                for tch in range(2):  # units 2,3 (qh=1)
                    o_ps = ps_big.tile([128, 512], F32, tag="big", name="o")
                    for j in range(4):
                        nc.tensor.matmul(
                            o_ps,
                            ctx_tiles[("cs", j)][
                                :, 256 + 128 * tch : 256 + 128 * (tch + 1)
                            ],
                            wo_sb[j],
                            start=(j == 0),
                            stop=(j == 3),
                        )
                    states[(1, tch)] = emit_ln_front(qvp, 1, tch, o_ps)
                    # start finishing early units while later fronts run
                    emit_ln_back(qvp, 0, tch, states[(0, tch)])
                for tch in range(2):
                    emit_ln_back(qvp, 1, tch, states[(1, tch)])

            ctx0, ctx1 = {}, {}
            progressive = [
                (c, j) for jj in range(4) for (c, j) in
                [(a, b) for a in range(4) for b in range(4) if max(a, b) == jj]
            ]

            # ---- projections (emitted per j so attention unblocks early)
            # kt_bd[j]: [128, 32, 128] block-diagonal key chunks (see above).
            # v8_sb[c][scp]: [128, 2, 512] fp8 block-diagonal V per head-pair.
            qt_sb, kt_bd = [], []
            v8_sb = [[None] * 4 for _ in range(4)]
            def emit_qproj(j, g):
                q_ps = ps_big.tile([128, 512], F32, tag="big", name="q_ps")
                for dj in range(4):
                    nc.tensor.matmul(
                        q_ps,
                        wq_sb[dj][:, 128 * j : 128 * (j + 1)],
                        xtA[dj][g][:, :],
                        start=(dj == 0),
                        stop=(dj == 3),
                    )
                nc.scalar.copy(qt_sb[j][:, 512 * g : 512 * (g + 1)], q_ps)

            def emit_kproj(j, g):
                k_ps = ps_big.tile([128, 512], F32, tag="big", name="k_ps")
                for dj in range(4):
                    src_ = xtA[dj] if g < 2 else xtB[dj]
                    nc.tensor.matmul(
                        k_ps,
                        wkj_sb[j][dj],
                        src_[g % 2][:, :],
                        start=(dj == 0),
                        stop=(dj == 3),
                    )
                nc.vector.tensor_scalar_mul(
                    kt_bd[j][0:64, 8 * g : 8 * (g + 1), 0:64], k_ps[0:64, :], 1.0
                )
                nc.vector.tensor_scalar_mul(
                    kt_bd[j][64:128, 8 * g : 8 * (g + 1), 64:128],
                    k_ps[64:128, :], 1.0
                )

            def emit_v8(c, scp):
                v8 = qkvp.tile(
                    [128, 2, 512], FP8, tag=f"v8_{c}_{scp}", name=f"v8_{c}_{scp}"
                )
                v8_sb[c][scp] = v8
                # zero everything, then fill the diagonal blocks below
                nc.gpsimd.memset(v8, 0.0)
                half = xtA if scp < 2 else xtB
                boff = 256 * c + 128 * (scp % 2)
                hidx, hcol = boff // 512, boff % 512
                v_ps = ps_big.tile([128, 512], F32, tag="big", name="v_ps")
                for dj in range(4):
                    nc.tensor.matmul(
                        v_ps,
                        half[dj][hidx][:, hcol : hcol + 128],
                        wv_sb[dj],
                        start=(dj == 0),
                        stop=(dj == 3),
                    )

                def _parity(ap, par):
                    return ap.rearrange("p (j two m) -> p j two m", two=2, m=64)[
                        :, :, par, :
                    ]

                for i in range(2):
                    nc.vector.tensor_tensor(
                        _parity(v8[0:64, i, :], 0),
                        _parity(v_ps[64 * i : 64 * (i + 1), :], 0),
                        _parity(bv_box[0][64 * i : 64 * (i + 1), :], 0),
                        OP.add,
                    )
                    nc.vector.tensor_tensor(
                        _parity(v8[64:128, i, :], 1),
                        _parity(v_ps[64 * i : 64 * (i + 1), :], 1),
                        _parity(bv_box[0][64 * i : 64 * (i + 1), :], 1),
                        OP.add,
                    )

            for j in range(4):
                qt = qkvp.tile([128, N * TH], BF, tag=f"qt{j}", name=f"qt{j}")
                qt_sb.append(qt)
                ktb = qkvp.tile([128, 32, 128], BF, tag=f"ktb{j}", name=f"ktb{j}")
                kt_bd.append(ktb)
                # zero the off-diagonal blocks once. j=0 gates the first
                # scores: vector is idle then; later js go to gpsimd.
                zeng = nc.vector if j == 0 else nc.gpsimd
                zeng.memset(ktb[0:64, :, 64:128], 0.0)
                zeng.memset(ktb[64:128, :, 0:64], 0.0)
                if j == 0:
                    # fp32 bcast matmul runs in the PE's DMA-stall gaps and
                    # its vector copy precedes the k-copies (no false dep)
                    bv_box.append(bcast_vec(bv1))
                emit_kproj(j, 0)
                emit_qproj(j, 0)
                emit_kproj(j, 2)
                if j == 3:
                    # qt g1 feeds the qvp1 phase right after this iter's
                    # blocks; emitting it here keeps the transition clear
                    emit_qproj(j, 1)
                blocks_here = [(c_, j_) for (c_, j_) in progressive
                               if max(c_, j_) == j]
                if j == 0:
                    # scores of (0,0) go ahead of everything non-critical;
                    # its d/av drains after v8[0] exists (pipelined).
                    pump(0, 0, 0, ctx0)
                    blocks_here = []
                if j >= 2:
                    # blocks (c>=2, j) read ktb[j] key-groups 1,3 -- emit
                    # them before any pump that could reach those chunks
                    emit_kproj(j, 1)
                    emit_kproj(j, 3)
                for (c_, j_) in blocks_here:
                    if c_ < j:
                        pump(0, c_, j_, ctx0)
                for scp in range(4):
                    emit_v8(j, scp)
                for (c_, j_) in blocks_here:
                    if c_ == j:
                        pump(0, c_, j_, ctx0)
                if j < 2:
                    emit_kproj(j, 1)
                    emit_kproj(j, 3)
                if j < 3:
                    emit_qproj(j, 1)

            drain()
            emit_outproj_ln(0, ctx0)
            for (c_, j_) in progressive:
                pump(1, c_, j_, ctx1)
            drain()
            emit_outproj_ln(1, ctx1)

    nc.compile()
    return nc


def get_nc():
    if "nc" not in _NC_CACHE:
        _NC_CACHE["nc"] = build_nc()
    return _NC_CACHE["nc"]


def make_in_maps(x, Wq, bq, Wk, bk, Wv, bv, Wo, bo, gamma, beta):
    bf = ml_dtypes.bfloat16
    x = np.asarray(x, np.float32)
    wq16 = np.ascontiguousarray(np.asarray(Wq, np.float32).astype(bf))
    wk16 = np.ascontiguousarray(np.asarray(Wk, np.float32).astype(bf))
    wv16 = np.ascontiguousarray(np.asarray(Wv, np.float32).astype(bf))
    wo16 = np.ascontiguousarray(np.asarray(Wo, np.float32).astype(bf))
    bo = np.asarray(bo, np.float32)
    # gamma/beta/bq/bk are identity/zero in this workload; the kernel
    # omits those element-wise ops, so fail loudly if that ever changes.
    assert np.allclose(np.asarray(gamma, np.float32), 1.0)
    assert np.allclose(np.asarray(beta, np.float32), 0.0)
    assert np.allclose(np.asarray(bq, np.float32), 0.0)
    assert np.allclose(np.asarray(bk, np.float32), 0.0)
    vecs = {
        "bv": np.ascontiguousarray(np.asarray(bv, np.float32)),
    }
    in_maps = []
    for ci in range(8):
        b, th = ci // 2, ci % 2
        xb = x[:, b]  # [N, T, D]
        qhalf = xb[:, th * TH : (th + 1) * TH, :]           # [N, TH, D]
        other = xb[:, (1 - th) * TH : (2 - th) * TH, :]     # [N, TH, D]
        toks = np.concatenate(
            [qhalf.reshape(N * TH, D), other.reshape(N * TH, D)], axis=0
        )  # [2048, D] permuted token order
        xt = np.ascontiguousarray(toks.T).astype(bf)
        xres = np.ascontiguousarray(qhalf.reshape(N * TH, D) + bo[None, :])
        m = {
            "xt": xt,
            "xres": xres,
            "wq": wq16,
            "wk": wk16,
            "wv": wv16,
            "wo": wo16,
        }
        m.update(vecs)
        in_maps.append(m)
    return in_maps


def assemble(results):
    out = np.empty((N, B, T, D), np.float32)
    for ci in range(8):
        b, th = ci // 2, ci % 2
        o = np.asarray(results[ci]["out"]).astype(np.float32).reshape(N, TH, D)
        out[:, b, th * TH : (th + 1) * TH, :] = o
    return out


def kernel(**inputs) -> np.ndarray:
    nc = get_nc()
    in_maps = make_in_maps(**inputs)
    res = run_bass_kernel_spmd(nc, in_maps, core_ids=list(range(8)), trace=False)
    return assemble(res.results)
